# revision 33
# baseline (speedup 1.0000x reference)
"""Bass/Tile TRN2 kernel: multi-head attention with a local (sliding-window)
causal mask, window = 128, fp16 compute with fp32 PSUM accumulation.

Problem: x[2, 4096, 1024], 16 heads x 64 dims, out = attn(x) @ Wo^T.

Sharding (8 cores): core c handles batch b = c // 4 and the 4 heads
h in [4*(c%4), 4*(c%4)+4). Each core computes its q/k/v projections
(256 output dims), local attention, and a partial output projection
[4096, 1024] over its 256 contraction dims. The host sums the 4 partials
per batch and adds the (softmax + 1e-9) rank-1 correction plus biases.

v2 layout: scores are computed TRANSPOSED (sT[j,i] = kT_blk.T @ qT_cols)
so the exp'd probability tiles feed the PV matmul directly as the
stationary operand -- no PE transposes of P needed. Probability storage
is keyed by query block (pmI[ib] = [j-block ib-1 | j-block ib] halves),
masks are constant triangles applied on GpSimd, softmax denominator
comes from a ones-column in V, and normalization is one broadcast
tensor_tensor. Outputs are cast to fp16 and summed on host.

Device layouts per core:
  qT/kT  [dk_on_partitions, seq]
  v      [j_on_partitions, 33 blocks x 4*(64+2)]  (block 0 zero; col 64 of
                                  each head group is 1.0 -> denominator)
  sT     [j_on_partitions, head, 256 i-cols] psum; exp -> pmI fp16
  ctx    [i, 4*65] psum -> normalized fp16 -> xbar transpose -> out proj
"""

import numpy as np
from contextlib import ExitStack

D_MODEL = 1024
SEQ = 4096
BATCH = 2
D_K = 64
O = 256            # head dims per core (4 heads x 64)
WIN = 128
SCALE = 0.125      # 1/sqrt(64)
N_CORES = 8
NB = SEQ // 128    # 32 query/key blocks
NST = SEQ // 512   # 8 projection column tiles

_CACHE = {}


def _build_program():
    import concourse.tile as tile
    from concourse import bacc, mybir

    f16 = mybir.dt.float16
    f32 = mybir.dt.float32
    AF = mybir.ActivationFunctionType

    nc = bacc.Bacc("TRN2", target_bir_lowering=False, debug=False,
                   num_devices=N_CORES)

    # all inputs host-pre-tiled so every DMA is contiguous (8KB lines)
    xt_d = nc.dram_tensor("xt", [NST, 128, 8, 512], f16,
                          kind="ExternalInput").ap()
    wq_d = nc.dram_tensor("wq", [128, 8, O], f16, kind="ExternalInput").ap()
    wk_d = nc.dram_tensor("wk", [128, 8, O], f16, kind="ExternalInput").ap()
    wv_d = nc.dram_tensor("wv", [128, 8, O], f16, kind="ExternalInput").ap()
    wo_d = nc.dram_tensor("wo", [128, 2, D_MODEL], f16,
                          kind="ExternalInput").ap()
    mi_d = nc.dram_tensor("maskin", [128, 256], f16, kind="ExternalInput").ap()
    out_d = nc.dram_tensor("out", [NB, 2, 128, 512], f16,
                           kind="ExternalOutput").ap()

    with tile.TileContext(nc) as tc, ExitStack() as ctx:
        consts = ctx.enter_context(tc.tile_pool(name="consts", bufs=1))
        store = ctx.enter_context(tc.tile_pool(name="store", bufs=1))
        xts = ctx.enter_context(tc.tile_pool(name="xts", bufs=2))
        pmes = ctx.enter_context(tc.tile_pool(name="pmes", bufs=4))
        pmms = ctx.enter_context(tc.tile_pool(name="pmms", bufs=3))
        cns = ctx.enter_context(tc.tile_pool(name="cns", bufs=3))
        cts = ctx.enter_context(tc.tile_pool(name="cts", bufs=3))
        recs = ctx.enter_context(tc.tile_pool(name="recs", bufs=3))
        outs = ctx.enter_context(tc.tile_pool(name="outs", bufs=6))
        # PSUM: bank-granular per (tag, buf); total must be <= 8 banks.
        pp = ctx.enter_context(tc.tile_pool(name="pp", bufs=2, space="PSUM"))
        pst = ctx.enter_context(tc.tile_pool(name="pst", bufs=2, space="PSUM"))
        pc = ctx.enter_context(tc.tile_pool(name="pc", bufs=1, space="PSUM"))

        # ---- constants (wq + first x tile first: critical path) ----
        # wq on the sync queue, x tiles on the scalar queue -> parallel
        # rings; first tiles chunked so the first matmuls start sooner.
        wq_sb = consts.tile([128, 8, O], f16)
        wk_sb = consts.tile([128, 8, O], f16)
        wv_sb = consts.tile([128, 8, O], f16)
        nc.sync.dma_start(out=wq_sb[:, 0:2], in_=wq_d[:, 0:2])
        xt0 = xts.tile([128, 8, 512], f16, tag="xt", name="xt0")
        nc.scalar.dma_start(out=xt0[:, 0:2], in_=xt_d[0, :, 0:2])
        nc.gpsimd.dma_start(out=wk_sb, in_=wk_d)
        nc.sync.dma_start(out=wq_sb[:, 2:8], in_=wq_d[:, 2:8])
        nc.scalar.dma_start(out=xt0[:, 2:4], in_=xt_d[0, :, 2:4])
        nc.scalar.dma_start(out=xt0[:, 4:8], in_=xt_d[0, :, 4:8])
        nc.gpsimd.dma_start(out=wv_sb, in_=wv_d)
        wo_sb = consts.tile([128, 2, D_MODEL], f16)
        nc.gpsimd.dma_start(out=wo_sb, in_=wo_d)
        mi_sb = consts.tile([128, 256], f16)
        nc.gpsimd.dma_start(out=mi_sb, in_=mi_d)
        ident = consts.tile([128, 128], f16)
        from concourse.masks import make_identity
        make_identity(nc, ident)

        qT = store.tile([128, 2, SEQ], f16)
        kT = store.tile([128, 2, SEQ], f16)
        # v blocks: index 0 is an all-zero block (j-block "-1" for ib=0);
        # col 64 of each head group is 1.0 -> PV emits the denominator.
        v = store.tile([128, NB + 1, 4 * (D_K + 2)], f16)
        v4 = v.rearrange("p j (h e) -> p j h e", e=D_K + 2)
        nc.vector.memset(v[:, 0, :], 0.0)
        for h in range(4):
            nc.vector.memset(v4[:, 1:NB + 1, h, D_K:D_K + 1], 1.0)

        # pmE[jb][:, s, 0:128]   = raw exp scores, i-block jb, j-block jb
        # pmE[jb][:, s, 128:256] = raw exp scores, i-block jb+1, j-block jb
        # slot s = device head order (h0, h2, h1, h3); host permutes
        # wv cols / wo rows to match, qT/kT layout already matches.
        # pmM[ib][:, s, 0:128]   = masked probs, j-block ib-1 (a=0)
        # pmM[ib][:, s, 128:256] = masked probs, j-block ib   (a=1)
        pmE = {}

        def proj_tile(st):
            s0 = st * 512
            if st == 0:
                xt = xt0
            else:
                xt = xts.tile([128, 8, 512], f16, tag="xt",
                              name=f"xtt{st}")
                nc.scalar.dma_start(out=xt, in_=xt_d[st])
            for w_sb, dst in ((wq_sb, qT), (wk_sb, kT)):
                for ot in range(2):
                    ps = pp.tile([128, 512], f32, tag="pp",
                                 name=f"psq{st}{ot}")
                    for dc in range(8):
                        nc.tensor.matmul(
                            ps,
                            lhsT=w_sb[:, dc, ot * 128:(ot + 1) * 128],
                            rhs=xt[:, dc, :],
                            start=(dc == 0), stop=(dc == 7))
                    nc.scalar.copy(out=dst[:, ot, s0:s0 + 512], in_=ps)
            for ss in range(4):
                jb = st * 4 + ss
                ps = pp.tile([128, 512], f32, tag="pp", name=f"psv{jb}")
                for dc in range(8):
                    nc.tensor.matmul(
                        ps[:, 0:O],
                        lhsT=xt[:, dc, ss * 128:(ss + 1) * 128],
                        rhs=wv_sb[:, dc, :],
                        start=(dc == 0), stop=(dc == 7))
                nc.vector.tensor_copy(
                    out=v4[:, 1 + jb, :, 0:D_K],
                    in_=ps[:, 0:O].rearrange("p (h e) -> p h e", e=D_K))

        pmMs = {}

        def attn_front(jb):
            """scores + exp + masks for block jb -> pmMs[jb]"""
            i0 = jb * 128
            ncols = 256 if jb < NB - 1 else 128
            stA = pst.tile([128, 2, 256], f32, tag="stA", name=f"stA{jb}")
            stB = pst.tile([128, 2, 256], f32, tag="stB", name=f"stB{jb}")
            for h in range(4):
                g, p0 = h // 2, (h % 2) * 64
                t_ = stA if h % 2 == 0 else stB
                nc.tensor.matmul(
                    t_[:, h // 2, 0:ncols],
                    lhsT=kT[p0:p0 + 64, g, i0:i0 + 128],
                    rhs=qT[p0:p0 + 64, g, i0:i0 + ncols],
                    start=True, stop=True)
            # exp: psum f32 -> pmE fp16 (dense per st tile)
            pmE[jb] = pmes.tile([128, 4, 256], f16, tag="pme",
                                name=f"pme{jb}")
            nc.scalar.activation(out=pmE[jb][:, 0:2, 0:ncols],
                                 in_=stA[:, :, 0:ncols], func=AF.Exp)
            nc.scalar.activation(out=pmE[jb][:, 2:4, 0:ncols],
                                 in_=stB[:, :, 0:ncols], func=AF.Exp)

            # masks: constant triangle bands, broadcast across slots
            ib = jb
            pmM = pmms.tile([128, 4, 256], f16, tag="pmm",
                            name=f"pmm{jb}")
            pmMs[jb] = pmM
            if ib > 0:
                nc.gpsimd.tensor_mul(
                    pmM[:, :, 0:128],
                    pmE[ib - 1][:, :, 128:256],
                    mi_sb[:, None, 0:128].broadcast_to([128, 4, 128]))
            nc.vector.tensor_mul(
                pmM[:, :, 128:256],
                pmE[ib][:, :, 0:128],
                mi_sb[:, None, 128:256].broadcast_to([128, 4, 128]))

        def attn_back(ib):
            """PV + normalize + transpose + out-proj for block ib"""
            i0 = ib * 128
            jb = ib
            pmM = pmMs.pop(ib)
            # PV: ctx[i, 65] per head; col 64 = softmax denominator
            cps = pc.tile([128, 4, D_K + 1], f32, tag="cps",
                          name=f"cps{jb}")
            alist = [1] if ib == 0 else [0, 1]
            for s in range(4):
                for idx, a in enumerate(alist):
                    nc.tensor.matmul(
                        cps[:, s, :],
                        lhsT=pmM[:, s, a * 128:(a + 1) * 128],
                        rhs=v4[:, ib + a, s, 0:D_K + 1],
                        start=(idx == 0), stop=(idx == len(alist) - 1))

            # normalize: one broadcast TT (cn layout: slot-major 4x64)
            rec4 = recs.tile([128, 4], f32, tag="rec", name=f"rec{jb}")
            nc.vector.reciprocal(
                rec4, cps[:, :, D_K:D_K + 1].rearrange("p s one -> p (s one)"))
            cn = cns.tile([128, 2, 128], f16, tag="cn", name=f"cn{jb}")
            nc.vector.tensor_mul(
                cn.rearrange("p a i -> p (a i)").rearrange(
                    "p (s e) -> p s e", e=D_K),
                cps[:, :, 0:D_K],
                rec4[:, :, None].broadcast_to([128, 4, D_K]))

            # transpose ctx -> [d, i] for the output projection
            ctp = pc.tile([128, 256], f16, tag="ctp", name=f"ctp{jb}")
            for cc in range(2):
                nc.tensor.transpose(
                    ctp[:, cc * 128:(cc + 1) * 128], cn[:, cc, :], ident)
            ct = cts.tile([128, 2, 128], f16, tag="ct", name=f"ct{jb}")
            nc.vector.tensor_copy(out=ct.rearrange("p a i -> p (a i)"), in_=ctp)

            for mh in range(2):
                po = pp.tile([128, 512], f32, tag="pp", name=f"po{jb}{mh}")
                for cc in range(2):
                    nc.tensor.matmul(
                        po,
                        lhsT=ct[:, cc, :],
                        rhs=wo_sb[:, cc, mh * 512:(mh + 1) * 512],
                        start=(cc == 0), stop=(cc == 1))
                ob = outs.tile([128, 512], f16, tag="ob", name=f"ob{jb}{mh}")
                cp = nc.scalar.copy if mh == 0 else nc.vector.tensor_copy
                cp(out=ob, in_=po)
                dma_eng = nc.sync if mh == 0 else nc.gpsimd
                dma_eng.dma_start(out=out_d[jb, mh], in_=ob)

        # phase-separated: dense projections keep the PE warm, then dense
        # attention (proj/attn interleaving measured slower: HAM
        # oscillation). The attention loop is software-pipelined one block
        # deep so PV never waits on the same block's exp+mask chain.
        for st in range(NST):
            proj_tile(st)
        for jb in range(NB):
            attn_front(jb)
            if jb >= 1:
                attn_back(jb - 1)
        attn_back(NB - 1)
    nc.compile()
    return nc


def get_program():
    if "nc" not in _CACHE:
        _CACHE["nc"] = _build_program()
    return _CACHE["nc"]


def _mask():
    # pmM column layout: [a=0 (j-block ib-1) | a=1 (j-block ib)].
    # a=0 (previous block): allowed iff j >= i; a=1 (same block): j <= i.
    j = np.arange(128)[:, None]
    i = np.arange(128)[None, :]
    a0 = (j >= i).astype(np.float16)
    a1 = (j <= i).astype(np.float16)
    return np.concatenate([a0, a1], axis=1)  # [128, 256]


# device slot order: slot s holds head HS[s] of this core's 4 heads
HS = [0, 2, 1, 3]
_SLOT_PERM = np.concatenate([np.arange(h * D_K, (h + 1) * D_K) for h in HS])


def _tile_w(w):
    # [1024, 256] -> [128, 8, 256] (partition-major, contiguous DMA lines)
    return np.ascontiguousarray(
        w.reshape(8, 128, -1).transpose(1, 0, 2)).astype(np.float16)


def make_in_maps(inputs):
    x = np.asarray(inputs["x"], np.float32)
    Wq = np.asarray(inputs["Wq"], np.float32)
    Wk = np.asarray(inputs["Wk"], np.float32)
    Wv = np.asarray(inputs["Wv"], np.float32)
    Wo = np.asarray(inputs["Wo"], np.float32)
    MI = _mask()
    in_maps = []
    for core in range(N_CORES):
        b, g = core // 4, core % 4
        sl = slice(g * O, (g + 1) * O)
        # x[b].T [1024, 4096] -> [st 8, p 128, a 8, s 512] contiguous
        xt = x[b].T.reshape(8, 128, 8, 512).transpose(2, 1, 0, 3)
        # wv cols / wo rows permuted to the device slot order
        wv = Wv[sl].T[:, _SLOT_PERM]
        wo = Wo[:, sl].T[_SLOT_PERM, :]
        in_maps.append({
            "xt": np.ascontiguousarray(xt).astype(np.float16),
            "wq": _tile_w(Wq[sl].T * SCALE),
            "wk": _tile_w(Wk[sl].T),
            "wv": _tile_w(wv),
            "wo": np.ascontiguousarray(
                wo.reshape(2, 128, 1024).transpose(1, 0, 2)
            ).astype(np.float16),
            "maskin": MI,
        })
    return in_maps


def combine(results, inputs):
    """Sum per-core partials and add host-side corrections."""
    x = np.asarray(inputs["x"], np.float32)
    Wv = np.asarray(inputs["Wv"], np.float32)
    Wo = np.asarray(inputs["Wo"], np.float32)
    bv = np.asarray(inputs["bv"], np.float32)
    bo = np.asarray(inputs["bo"], np.float32)
    out = np.zeros((BATCH, SEQ, D_MODEL), np.float32)
    for core in range(N_CORES):
        # device layout [ib, mh, 128, 512] -> [4096, 1024]
        o = results[core]["out"].astype(np.float32)
        out[core // 4] += o.transpose(0, 2, 1, 3).reshape(SEQ, D_MODEL)
    # reference adds 1e-9 to every attn prob (including masked ones):
    # ctx += 1e-9 * sum_j v[j]  ->  out += 1e-9 * (sum_j v[j]) @ Wo^T
    for b in range(BATCH):
        vs = x[b].sum(axis=0) @ Wv.T + SEQ * bv
        out[b] += (1e-9 * (vs @ Wo.T) + bo)[None, :]
    return out


def run_cores(in_maps, trace=False, **kw):
    from concourse.bass_utils import run_bass_kernel_spmd
    nc = get_program()
    return run_bass_kernel_spmd(nc, in_maps, core_ids=list(range(N_CORES)),
                                trace=trace, **kw)


def kernel(**inputs):
    in_maps = make_in_maps(inputs)
    res = run_cores(in_maps)
    return combine(res.results, inputs)


# revision 34
# speedup vs baseline: 1.0064x; 1.0064x over previous
"""Bass/Tile TRN2 kernel: multi-head attention with a local (sliding-window)
causal mask, window = 128, fp16 compute with fp32 PSUM accumulation.

Problem: x[2, 4096, 1024], 16 heads x 64 dims, out = attn(x) @ Wo^T.

Sharding (8 cores): core c handles batch b = c // 4 and the 4 heads
h in [4*(c%4), 4*(c%4)+4). Each core computes its q/k/v projections
(256 output dims), local attention, and a partial output projection
[4096, 1024] over its 256 contraction dims. The host sums the 4 partials
per batch and adds the (softmax + 1e-9) rank-1 correction plus biases.

v2 layout: scores are computed TRANSPOSED (sT[j,i] = kT_blk.T @ qT_cols)
so the exp'd probability tiles feed the PV matmul directly as the
stationary operand -- no PE transposes of P needed. Probability storage
is keyed by query block (pmI[ib] = [j-block ib-1 | j-block ib] halves),
masks are constant triangles applied on GpSimd, softmax denominator
comes from a ones-column in V, and normalization is one broadcast
tensor_tensor. Outputs are cast to fp16 and summed on host.

Device layouts per core:
  qT/kT  [dk_on_partitions, seq]
  v      [j_on_partitions, 33 blocks x 4*(64+2)]  (block 0 zero; col 64 of
                                  each head group is 1.0 -> denominator)
  sT     [j_on_partitions, head, 256 i-cols] psum; exp -> pmI fp16
  ctx    [i, 4*65] psum -> normalized fp16 -> xbar transpose -> out proj
"""

import numpy as np
from contextlib import ExitStack

D_MODEL = 1024
SEQ = 4096
BATCH = 2
D_K = 64
O = 256            # head dims per core (4 heads x 64)
WIN = 128
SCALE = 0.125      # 1/sqrt(64)
N_CORES = 8
NB = SEQ // 128    # 32 query/key blocks
NST = SEQ // 512   # 8 projection column tiles

_CACHE = {}


def _build_program():
    import concourse.tile as tile
    from concourse import bacc, mybir

    f16 = mybir.dt.float16
    f32 = mybir.dt.float32
    AF = mybir.ActivationFunctionType

    nc = bacc.Bacc("TRN2", target_bir_lowering=False, debug=False,
                   num_devices=N_CORES)

    # all inputs host-pre-tiled so every DMA is contiguous (8KB lines)
    xt_d = nc.dram_tensor("xt", [NST, 128, 8, 512], f16,
                          kind="ExternalInput").ap()
    wq_d = nc.dram_tensor("wq", [128, 8, O], f16, kind="ExternalInput").ap()
    wk_d = nc.dram_tensor("wk", [128, 8, O], f16, kind="ExternalInput").ap()
    wv_d = nc.dram_tensor("wv", [128, 8, O], f16, kind="ExternalInput").ap()
    wo_d = nc.dram_tensor("wo", [128, 2, D_MODEL], f16,
                          kind="ExternalInput").ap()
    mi_d = nc.dram_tensor("maskin", [128, 256], f16, kind="ExternalInput").ap()
    out_d = nc.dram_tensor("out", [NB, 2, 128, 512], f16,
                           kind="ExternalOutput").ap()

    with tile.TileContext(nc) as tc, ExitStack() as ctx:
        consts = ctx.enter_context(tc.tile_pool(name="consts", bufs=1))
        store = ctx.enter_context(tc.tile_pool(name="store", bufs=1))
        xts = ctx.enter_context(tc.tile_pool(name="xts", bufs=2))
        pmes = ctx.enter_context(tc.tile_pool(name="pmes", bufs=4))
        pmms = ctx.enter_context(tc.tile_pool(name="pmms", bufs=3))
        cns = ctx.enter_context(tc.tile_pool(name="cns", bufs=3))
        cts = ctx.enter_context(tc.tile_pool(name="cts", bufs=3))
        recs = ctx.enter_context(tc.tile_pool(name="recs", bufs=3))
        outs = ctx.enter_context(tc.tile_pool(name="outs", bufs=6))
        # PSUM: bank-granular per (tag, buf); total must be <= 8 banks.
        pp = ctx.enter_context(tc.tile_pool(name="pp", bufs=2, space="PSUM"))
        pst = ctx.enter_context(tc.tile_pool(name="pst", bufs=2, space="PSUM"))
        pc = ctx.enter_context(tc.tile_pool(name="pc", bufs=1, space="PSUM"))

        # ---- constants (wq + first x tile first: critical path) ----
        # wq on the sync queue, x tiles on the scalar queue -> parallel
        # rings; first tiles chunked so the first matmuls start sooner.
        wq_sb = consts.tile([128, 8, O], f16)
        wk_sb = consts.tile([128, 8, O], f16)
        wv_sb = consts.tile([128, 8, O], f16)
        nc.sync.dma_start(out=wq_sb[:, 0:2], in_=wq_d[:, 0:2])
        xt0 = xts.tile([128, 8, 512], f16, tag="xt", name="xt0")
        nc.scalar.dma_start(out=xt0[:, 0:2], in_=xt_d[0, :, 0:2])
        nc.gpsimd.dma_start(out=wk_sb, in_=wk_d)
        nc.sync.dma_start(out=wq_sb[:, 2:8], in_=wq_d[:, 2:8])
        nc.scalar.dma_start(out=xt0[:, 2:4], in_=xt_d[0, :, 2:4])
        nc.scalar.dma_start(out=xt0[:, 4:8], in_=xt_d[0, :, 4:8])
        nc.gpsimd.dma_start(out=wv_sb, in_=wv_d)
        wo_sb = consts.tile([128, 2, D_MODEL], f16)
        nc.gpsimd.dma_start(out=wo_sb, in_=wo_d)
        mi_sb = consts.tile([128, 256], f16)
        nc.gpsimd.dma_start(out=mi_sb, in_=mi_d)
        ident = consts.tile([128, 128], f16)
        from concourse.masks import make_identity
        make_identity(nc, ident)

        qT = store.tile([128, 2, SEQ], f16)
        kT = store.tile([128, 2, SEQ], f16)
        # v blocks: index 0 is an all-zero block (j-block "-1" for ib=0);
        # col 64 of each head group is 1.0 -> PV emits the denominator.
        v = store.tile([128, NB + 1, 4 * (D_K + 2)], f16)
        v4 = v.rearrange("p j (h e) -> p j h e", e=D_K + 2)
        nc.vector.memset(v[:, 0, :], 0.0)
        for h in range(4):
            nc.vector.memset(v4[:, 1:NB + 1, h, D_K:D_K + 1], 1.0)

        # pmE[jb][:, s, 0:128]   = raw exp scores, i-block jb, j-block jb
        # pmE[jb][:, s, 128:256] = raw exp scores, i-block jb+1, j-block jb
        # slot s = device head order (h0, h2, h1, h3); host permutes
        # wv cols / wo rows to match, qT/kT layout already matches.
        # pmM[ib][:, s, 0:128]   = masked probs, j-block ib-1 (a=0)
        # pmM[ib][:, s, 128:256] = masked probs, j-block ib   (a=1)
        pmE = {}

        def proj_tile(st):
            s0 = st * 512
            if st == 0:
                xt = xt0
            else:
                xt = xts.tile([128, 8, 512], f16, tag="xt",
                              name=f"xtt{st}")
                nc.scalar.dma_start(out=xt, in_=xt_d[st])
            for w_sb, dst in ((wq_sb, qT), (wk_sb, kT)):
                for ot in range(2):
                    ps = pp.tile([128, 512], f32, tag="pp",
                                 name=f"psq{st}{ot}")
                    for dc in range(8):
                        nc.tensor.matmul(
                            ps,
                            lhsT=w_sb[:, dc, ot * 128:(ot + 1) * 128],
                            rhs=xt[:, dc, :],
                            start=(dc == 0), stop=(dc == 7))
                    nc.scalar.copy(out=dst[:, ot, s0:s0 + 512], in_=ps)
            for ss in range(4):
                jb = st * 4 + ss
                ps = pp.tile([128, 512], f32, tag="pp", name=f"psv{jb}")
                for dc in range(8):
                    nc.tensor.matmul(
                        ps[:, 0:O],
                        lhsT=xt[:, dc, ss * 128:(ss + 1) * 128],
                        rhs=wv_sb[:, dc, :],
                        start=(dc == 0), stop=(dc == 7))
                nc.vector.tensor_copy(
                    out=v4[:, 1 + jb, :, 0:D_K],
                    in_=ps[:, 0:O].rearrange("p (h e) -> p h e", e=D_K))

        pmMs = {}

        def attn_front(jb):
            """scores + exp + masks for block jb -> pmMs[jb]"""
            i0 = jb * 128
            ncols = 256 if jb < NB - 1 else 128
            stA = pst.tile([128, 2, 256], f32, tag="stA", name=f"stA{jb}")
            stB = pst.tile([128, 2, 256], f32, tag="stB", name=f"stB{jb}")
            for h in range(4):
                g, p0 = h // 2, (h % 2) * 64
                t_ = stA if h % 2 == 0 else stB
                nc.tensor.matmul(
                    t_[:, h // 2, 0:ncols],
                    lhsT=kT[p0:p0 + 64, g, i0:i0 + 128],
                    rhs=qT[p0:p0 + 64, g, i0:i0 + ncols],
                    start=True, stop=True)
            # exp: psum f32 -> pmE fp16 (dense per st tile)
            pmE[jb] = pmes.tile([128, 4, 256], f16, tag="pme",
                                name=f"pme{jb}")
            nc.scalar.activation(out=pmE[jb][:, 0:2, 0:ncols],
                                 in_=stA[:, :, 0:ncols], func=AF.Exp)
            nc.scalar.activation(out=pmE[jb][:, 2:4, 0:ncols],
                                 in_=stB[:, :, 0:ncols], func=AF.Exp)

            # masks: constant triangle bands, broadcast across slots
            ib = jb
            pmM = pmms.tile([128, 4, 256], f16, tag="pmm",
                            name=f"pmm{jb}")
            pmMs[jb] = pmM
            if ib > 0:
                nc.gpsimd.tensor_mul(
                    pmM[:, :, 0:128],
                    pmE[ib - 1][:, :, 128:256],
                    mi_sb[:, None, 0:128].broadcast_to([128, 4, 128]))
            nc.vector.tensor_mul(
                pmM[:, :, 128:256],
                pmE[ib][:, :, 0:128],
                mi_sb[:, None, 128:256].broadcast_to([128, 4, 128]))

        def attn_back(ib):
            """PV + normalize + transpose + out-proj for block ib"""
            i0 = ib * 128
            jb = ib
            pmM = pmMs.pop(ib)
            # PV: ctx[i, 65] per head; col 64 = softmax denominator
            cps = pc.tile([128, 4, D_K + 1], f32, tag="cps",
                          name=f"cps{jb}")
            alist = [1] if ib == 0 else [0, 1]
            for s in range(4):
                for idx, a in enumerate(alist):
                    nc.tensor.matmul(
                        cps[:, s, :],
                        lhsT=pmM[:, s, a * 128:(a + 1) * 128],
                        rhs=v4[:, ib + a, s, 0:D_K + 1],
                        start=(idx == 0), stop=(idx == len(alist) - 1))

            # normalize: one broadcast TT (cn layout: slot-major 4x64)
            rec4 = recs.tile([128, 4], f32, tag="rec", name=f"rec{jb}")
            nc.vector.reciprocal(
                rec4, cps[:, :, D_K:D_K + 1].rearrange("p s one -> p (s one)"))
            cn = cns.tile([128, 2, 128], f16, tag="cn", name=f"cn{jb}")
            nc.vector.tensor_mul(
                cn.rearrange("p a i -> p (a i)").rearrange(
                    "p (s e) -> p s e", e=D_K),
                cps[:, :, 0:D_K],
                rec4[:, :, None].broadcast_to([128, 4, D_K]))

            # transpose ctx -> [d, i] for the output projection
            ctp = pc.tile([128, 256], f16, tag="ctp", name=f"ctp{jb}")
            for cc in range(2):
                nc.tensor.transpose(
                    ctp[:, cc * 128:(cc + 1) * 128], cn[:, cc, :], ident)
            ct = cts.tile([128, 2, 128], f16, tag="ct", name=f"ct{jb}")
            nc.vector.tensor_copy(out=ct.rearrange("p a i -> p (a i)"), in_=ctp)

            for mh in range(2):
                po = pp.tile([128, 512], f32, tag="pp", name=f"po{jb}{mh}")
                for cc in range(2):
                    nc.tensor.matmul(
                        po,
                        lhsT=ct[:, cc, :],
                        rhs=wo_sb[:, cc, mh * 512:(mh + 1) * 512],
                        start=(cc == 0), stop=(cc == 1))
                ob = outs.tile([128, 512], f16, tag="ob", name=f"ob{jb}{mh}")
                cp = nc.scalar.copy if mh == 0 else nc.vector.tensor_copy
                cp(out=ob, in_=po)
                nc.sync.dma_start(out=out_d[jb, mh], in_=ob)

        # phase-separated: dense projections keep the PE warm, then dense
        # attention (proj/attn interleaving measured slower: HAM
        # oscillation). The attention loop is software-pipelined one block
        # deep so PV never waits on the same block's exp+mask chain.
        for st in range(NST):
            proj_tile(st)
        for jb in range(NB):
            attn_front(jb)
            if jb >= 1:
                attn_back(jb - 1)
        attn_back(NB - 1)
    nc.compile()
    return nc


def get_program():
    if "nc" not in _CACHE:
        _CACHE["nc"] = _build_program()
    return _CACHE["nc"]


def _mask():
    # pmM column layout: [a=0 (j-block ib-1) | a=1 (j-block ib)].
    # a=0 (previous block): allowed iff j >= i; a=1 (same block): j <= i.
    j = np.arange(128)[:, None]
    i = np.arange(128)[None, :]
    a0 = (j >= i).astype(np.float16)
    a1 = (j <= i).astype(np.float16)
    return np.concatenate([a0, a1], axis=1)  # [128, 256]


# device slot order: slot s holds head HS[s] of this core's 4 heads
HS = [0, 2, 1, 3]
_SLOT_PERM = np.concatenate([np.arange(h * D_K, (h + 1) * D_K) for h in HS])


def _tile_w(w):
    # [1024, 256] -> [128, 8, 256] (partition-major, contiguous DMA lines)
    return np.ascontiguousarray(
        w.reshape(8, 128, -1).transpose(1, 0, 2)).astype(np.float16)


def make_in_maps(inputs):
    x = np.asarray(inputs["x"], np.float32)
    Wq = np.asarray(inputs["Wq"], np.float32)
    Wk = np.asarray(inputs["Wk"], np.float32)
    Wv = np.asarray(inputs["Wv"], np.float32)
    Wo = np.asarray(inputs["Wo"], np.float32)
    MI = _mask()
    in_maps = []
    for core in range(N_CORES):
        b, g = core // 4, core % 4
        sl = slice(g * O, (g + 1) * O)
        # x[b].T [1024, 4096] -> [st 8, p 128, a 8, s 512] contiguous
        xt = x[b].T.reshape(8, 128, 8, 512).transpose(2, 1, 0, 3)
        # wv cols / wo rows permuted to the device slot order
        wv = Wv[sl].T[:, _SLOT_PERM]
        wo = Wo[:, sl].T[_SLOT_PERM, :]
        in_maps.append({
            "xt": np.ascontiguousarray(xt).astype(np.float16),
            "wq": _tile_w(Wq[sl].T * SCALE),
            "wk": _tile_w(Wk[sl].T),
            "wv": _tile_w(wv),
            "wo": np.ascontiguousarray(
                wo.reshape(2, 128, 1024).transpose(1, 0, 2)
            ).astype(np.float16),
            "maskin": MI,
        })
    return in_maps


def combine(results, inputs):
    """Sum per-core partials and add host-side corrections."""
    x = np.asarray(inputs["x"], np.float32)
    Wv = np.asarray(inputs["Wv"], np.float32)
    Wo = np.asarray(inputs["Wo"], np.float32)
    bv = np.asarray(inputs["bv"], np.float32)
    bo = np.asarray(inputs["bo"], np.float32)
    out = np.zeros((BATCH, SEQ, D_MODEL), np.float32)
    for core in range(N_CORES):
        # device layout [ib, mh, 128, 512] -> [4096, 1024]
        o = results[core]["out"].astype(np.float32)
        out[core // 4] += o.transpose(0, 2, 1, 3).reshape(SEQ, D_MODEL)
    # reference adds 1e-9 to every attn prob (including masked ones):
    # ctx += 1e-9 * sum_j v[j]  ->  out += 1e-9 * (sum_j v[j]) @ Wo^T
    for b in range(BATCH):
        vs = x[b].sum(axis=0) @ Wv.T + SEQ * bv
        out[b] += (1e-9 * (vs @ Wo.T) + bo)[None, :]
    return out


def run_cores(in_maps, trace=False, **kw):
    from concourse.bass_utils import run_bass_kernel_spmd
    nc = get_program()
    return run_bass_kernel_spmd(nc, in_maps, core_ids=list(range(N_CORES)),
                                trace=trace, **kw)


def kernel(**inputs):
    in_maps = make_in_maps(inputs)
    res = run_cores(in_maps)
    return combine(res.results, inputs)


# revision 35
# speedup vs baseline: 1.0310x; 1.0245x over previous
"""Bass/Tile TRN2 kernel: multi-head attention with a local (sliding-window)
causal mask, window = 128, fp16 compute with fp32 PSUM accumulation.

Problem: x[2, 4096, 1024], 16 heads x 64 dims, out = attn(x) @ Wo^T.

Sharding (8 cores): core c handles batch b = c // 4 and the 4 heads
h in [4*(c%4), 4*(c%4)+4). Each core computes its q/k/v projections
(256 output dims), local attention, and a partial output projection
[4096, 1024] over its 256 contraction dims. The host sums the 4 partials
per batch and adds the (softmax + 1e-9) rank-1 correction plus biases.

v2 layout: scores are computed TRANSPOSED (sT[j,i] = kT_blk.T @ qT_cols)
so the exp'd probability tiles feed the PV matmul directly as the
stationary operand -- no PE transposes of P needed. Probability storage
is keyed by query block (pmI[ib] = [j-block ib-1 | j-block ib] halves),
masks are constant triangles applied on GpSimd, softmax denominator
comes from a ones-column in V, and normalization is one broadcast
tensor_tensor. Outputs are cast to fp16 and summed on host.

Device layouts per core:
  qT/kT  [dk_on_partitions, seq]
  v      [j_on_partitions, 33 blocks x 4*(64+2)]  (block 0 zero; col 64 of
                                  each head group is 1.0 -> denominator)
  sT     [j_on_partitions, head, 256 i-cols] psum; exp -> pmI fp16
  ctx    [i, 4*65] psum -> normalized fp16 -> xbar transpose -> out proj
"""

import numpy as np
from contextlib import ExitStack

D_MODEL = 1024
SEQ = 4096
BATCH = 2
D_K = 64
O = 256            # head dims per core (4 heads x 64)
WIN = 128
SCALE = 0.125      # 1/sqrt(64)
N_CORES = 8
NB = SEQ // 128    # 32 query/key blocks
NST = SEQ // 512   # 8 projection column tiles

_CACHE = {}


def _build_program():
    import concourse.tile as tile
    from concourse import bacc, mybir

    f16 = mybir.dt.float16
    f32 = mybir.dt.float32
    AF = mybir.ActivationFunctionType

    nc = bacc.Bacc("TRN2", target_bir_lowering=False, debug=False,
                   num_devices=N_CORES)

    # all inputs host-pre-tiled so every DMA is contiguous (8KB lines)
    xt_d = nc.dram_tensor("xt", [NST, 128, 8, 512], f16,
                          kind="ExternalInput").ap()
    wq_d = nc.dram_tensor("wq", [128, 8, O], f16, kind="ExternalInput").ap()
    wk_d = nc.dram_tensor("wk", [128, 8, O], f16, kind="ExternalInput").ap()
    wv_d = nc.dram_tensor("wv", [128, 8, O], f16, kind="ExternalInput").ap()
    wo_d = nc.dram_tensor("wo", [128, 2, D_MODEL], f16,
                          kind="ExternalInput").ap()
    mi_d = nc.dram_tensor("maskin", [128, 256], f16, kind="ExternalInput").ap()
    out_d = nc.dram_tensor("out", [NB, 2, 128, 512], f16,
                           kind="ExternalOutput").ap()

    with tile.TileContext(nc) as tc, ExitStack() as ctx:
        consts = ctx.enter_context(tc.tile_pool(name="consts", bufs=1))
        store = ctx.enter_context(tc.tile_pool(name="store", bufs=1))
        xts = ctx.enter_context(tc.tile_pool(name="xts", bufs=2))
        pmes = ctx.enter_context(tc.tile_pool(name="pmes", bufs=4))
        pmms = ctx.enter_context(tc.tile_pool(name="pmms", bufs=3))
        cns = ctx.enter_context(tc.tile_pool(name="cns", bufs=3))
        cts = ctx.enter_context(tc.tile_pool(name="cts", bufs=3))
        recs = ctx.enter_context(tc.tile_pool(name="recs", bufs=3))
        outs = ctx.enter_context(tc.tile_pool(name="outs", bufs=6))
        # PSUM: bank-granular per (tag, buf); total must be <= 8 banks.
        pp = ctx.enter_context(tc.tile_pool(name="pp", bufs=2, space="PSUM"))
        pst = ctx.enter_context(tc.tile_pool(name="pst", bufs=2, space="PSUM"))
        pc = ctx.enter_context(tc.tile_pool(name="pc", bufs=1, space="PSUM"))

        # ---- constants (wq + first x tile first: critical path) ----
        # wq on the sync queue, x tiles on the scalar queue -> parallel
        # rings; first tiles chunked so the first matmuls start sooner.
        wq_sb = consts.tile([128, 8, O], f16)
        wk_sb = consts.tile([128, 8, O], f16)
        wv_sb = consts.tile([128, 8, O], f16)
        nc.sync.dma_start(out=wq_sb[:, 0:2], in_=wq_d[:, 0:2])
        xt0 = xts.tile([128, 8, 512], f16, tag="xt", name="xt0")
        nc.scalar.dma_start(out=xt0[:, 0:2], in_=xt_d[0, :, 0:2])
        nc.sync.dma_start(out=wq_sb[:, 2:8], in_=wq_d[:, 2:8])
        nc.scalar.dma_start(out=xt0[:, 2:4], in_=xt_d[0, :, 2:4])
        nc.sync.dma_start(out=wk_sb[:, 0:4], in_=wk_d[:, 0:4])
        nc.scalar.dma_start(out=xt0[:, 4:8], in_=xt_d[0, :, 4:8])
        nc.sync.dma_start(out=wk_sb[:, 4:8], in_=wk_d[:, 4:8])
        nc.sync.dma_start(out=wv_sb, in_=wv_d)
        wo_sb = consts.tile([128, 2, D_MODEL], f16)
        nc.gpsimd.dma_start(out=wo_sb, in_=wo_d)
        mi_sb = consts.tile([128, 256], f16)
        nc.gpsimd.dma_start(out=mi_sb, in_=mi_d)
        ident = consts.tile([128, 128], f16)
        from concourse.masks import make_identity
        make_identity(nc, ident)

        qT = store.tile([128, 2, SEQ], f16)
        kT = store.tile([128, 2, SEQ], f16)
        # v blocks: index 0 is an all-zero block (j-block "-1" for ib=0);
        # col 64 of each head group is 1.0 -> PV emits the denominator.
        v = store.tile([128, NB + 1, 4 * (D_K + 2)], f16)
        v4 = v.rearrange("p j (h e) -> p j h e", e=D_K + 2)
        nc.vector.memset(v[:, 0, :], 0.0)
        for h in range(4):
            nc.vector.memset(v4[:, 1:NB + 1, h, D_K:D_K + 1], 1.0)

        # pmE[jb][:, s, 0:128]   = raw exp scores, i-block jb, j-block jb
        # pmE[jb][:, s, 128:256] = raw exp scores, i-block jb+1, j-block jb
        # slot s = device head order (h0, h2, h1, h3); host permutes
        # wv cols / wo rows to match, qT/kT layout already matches.
        # pmM[ib][:, s, 0:128]   = masked probs, j-block ib-1 (a=0)
        # pmM[ib][:, s, 128:256] = masked probs, j-block ib   (a=1)
        pmE = {}

        def proj_tile(st):
            s0 = st * 512
            if st == 0:
                xt = xt0
            else:
                xt = xts.tile([128, 8, 512], f16, tag="xt",
                              name=f"xtt{st}")
                nc.scalar.dma_start(out=xt, in_=xt_d[st])
            for w_sb, dst in ((wq_sb, qT), (wk_sb, kT)):
                for ot in range(2):
                    ps = pp.tile([128, 512], f32, tag="pp",
                                 name=f"psq{st}{ot}")
                    for dc in range(8):
                        nc.tensor.matmul(
                            ps,
                            lhsT=w_sb[:, dc, ot * 128:(ot + 1) * 128],
                            rhs=xt[:, dc, :],
                            start=(dc == 0), stop=(dc == 7))
                    nc.scalar.copy(out=dst[:, ot, s0:s0 + 512], in_=ps)
            for ss in range(4):
                jb = st * 4 + ss
                ps = pp.tile([128, 512], f32, tag="pp", name=f"psv{jb}")
                for dc in range(8):
                    nc.tensor.matmul(
                        ps[:, 0:O],
                        lhsT=xt[:, dc, ss * 128:(ss + 1) * 128],
                        rhs=wv_sb[:, dc, :],
                        start=(dc == 0), stop=(dc == 7))
                nc.vector.tensor_copy(
                    out=v4[:, 1 + jb, :, 0:D_K],
                    in_=ps[:, 0:O].rearrange("p (h e) -> p h e", e=D_K))

        pmMs = {}

        def attn_front(jb):
            """scores + exp + masks for block jb -> pmMs[jb]"""
            i0 = jb * 128
            ncols = 256 if jb < NB - 1 else 128
            stA = pst.tile([128, 2, 256], f32, tag="stA", name=f"stA{jb}")
            stB = pst.tile([128, 2, 256], f32, tag="stB", name=f"stB{jb}")
            for h in range(4):
                g, p0 = h // 2, (h % 2) * 64
                t_ = stA if h % 2 == 0 else stB
                nc.tensor.matmul(
                    t_[:, h // 2, 0:ncols],
                    lhsT=kT[p0:p0 + 64, g, i0:i0 + 128],
                    rhs=qT[p0:p0 + 64, g, i0:i0 + ncols],
                    start=True, stop=True)
            # exp: psum f32 -> pmE fp16 (dense per st tile)
            pmE[jb] = pmes.tile([128, 4, 256], f16, tag="pme",
                                name=f"pme{jb}")
            nc.scalar.activation(out=pmE[jb][:, 0:2, 0:ncols],
                                 in_=stA[:, :, 0:ncols], func=AF.Exp)
            nc.scalar.activation(out=pmE[jb][:, 2:4, 0:ncols],
                                 in_=stB[:, :, 0:ncols], func=AF.Exp)

            # masks: constant triangle bands, broadcast across slots
            ib = jb
            pmM = pmms.tile([128, 4, 256], f16, tag="pmm",
                            name=f"pmm{jb}")
            pmMs[jb] = pmM
            if ib > 0:
                nc.gpsimd.tensor_mul(
                    pmM[:, :, 0:128],
                    pmE[ib - 1][:, :, 128:256],
                    mi_sb[:, None, 0:128].broadcast_to([128, 4, 128]))
            nc.vector.tensor_mul(
                pmM[:, :, 128:256],
                pmE[ib][:, :, 0:128],
                mi_sb[:, None, 128:256].broadcast_to([128, 4, 128]))

        def attn_back(ib):
            """PV + normalize + transpose + out-proj for block ib"""
            i0 = ib * 128
            jb = ib
            pmM = pmMs.pop(ib)
            # PV: ctx[i, 65] per head; col 64 = softmax denominator
            cps = pc.tile([128, 4, D_K + 1], f32, tag="cps",
                          name=f"cps{jb}")
            alist = [1] if ib == 0 else [0, 1]
            for s in range(4):
                for idx, a in enumerate(alist):
                    nc.tensor.matmul(
                        cps[:, s, :],
                        lhsT=pmM[:, s, a * 128:(a + 1) * 128],
                        rhs=v4[:, ib + a, s, 0:D_K + 1],
                        start=(idx == 0), stop=(idx == len(alist) - 1))

            # normalize: one broadcast TT (cn layout: slot-major 4x64)
            rec4 = recs.tile([128, 4], f32, tag="rec", name=f"rec{jb}")
            nc.vector.reciprocal(
                rec4, cps[:, :, D_K:D_K + 1].rearrange("p s one -> p (s one)"))
            cn = cns.tile([128, 2, 128], f16, tag="cn", name=f"cn{jb}")
            nc.vector.tensor_mul(
                cn.rearrange("p a i -> p (a i)").rearrange(
                    "p (s e) -> p s e", e=D_K),
                cps[:, :, 0:D_K],
                rec4[:, :, None].broadcast_to([128, 4, D_K]))

            # transpose ctx -> [d, i] for the output projection
            ctp = pc.tile([128, 256], f16, tag="ctp", name=f"ctp{jb}")
            for cc in range(2):
                nc.tensor.transpose(
                    ctp[:, cc * 128:(cc + 1) * 128], cn[:, cc, :], ident)
            ct = cts.tile([128, 2, 128], f16, tag="ct", name=f"ct{jb}")
            nc.vector.tensor_copy(out=ct.rearrange("p a i -> p (a i)"), in_=ctp)

            for mh in range(2):
                po = pp.tile([128, 512], f32, tag="pp", name=f"po{jb}{mh}")
                for cc in range(2):
                    nc.tensor.matmul(
                        po,
                        lhsT=ct[:, cc, :],
                        rhs=wo_sb[:, cc, mh * 512:(mh + 1) * 512],
                        start=(cc == 0), stop=(cc == 1))
                ob = outs.tile([128, 512], f16, tag="ob", name=f"ob{jb}{mh}")
                cp = nc.scalar.copy if mh == 0 else nc.vector.tensor_copy
                cp(out=ob, in_=po)
                nc.sync.dma_start(out=out_d[jb, mh], in_=ob)

        # phase-separated: dense projections keep the PE warm, then dense
        # attention (proj/attn interleaving measured slower: HAM
        # oscillation). The attention loop is software-pipelined one block
        # deep so PV never waits on the same block's exp+mask chain.
        for st in range(NST):
            proj_tile(st)
        for jb in range(NB):
            attn_front(jb)
            if jb >= 1:
                attn_back(jb - 1)
        attn_back(NB - 1)
    nc.compile()
    return nc


def get_program():
    if "nc" not in _CACHE:
        _CACHE["nc"] = _build_program()
    return _CACHE["nc"]


def _mask():
    # pmM column layout: [a=0 (j-block ib-1) | a=1 (j-block ib)].
    # a=0 (previous block): allowed iff j >= i; a=1 (same block): j <= i.
    j = np.arange(128)[:, None]
    i = np.arange(128)[None, :]
    a0 = (j >= i).astype(np.float16)
    a1 = (j <= i).astype(np.float16)
    return np.concatenate([a0, a1], axis=1)  # [128, 256]


# device slot order: slot s holds head HS[s] of this core's 4 heads
HS = [0, 2, 1, 3]
_SLOT_PERM = np.concatenate([np.arange(h * D_K, (h + 1) * D_K) for h in HS])


def _tile_w(w):
    # [1024, 256] -> [128, 8, 256] (partition-major, contiguous DMA lines)
    return np.ascontiguousarray(
        w.reshape(8, 128, -1).transpose(1, 0, 2)).astype(np.float16)


def make_in_maps(inputs):
    x = np.asarray(inputs["x"], np.float32)
    Wq = np.asarray(inputs["Wq"], np.float32)
    Wk = np.asarray(inputs["Wk"], np.float32)
    Wv = np.asarray(inputs["Wv"], np.float32)
    Wo = np.asarray(inputs["Wo"], np.float32)
    MI = _mask()
    in_maps = []
    for core in range(N_CORES):
        b, g = core // 4, core % 4
        sl = slice(g * O, (g + 1) * O)
        # x[b].T [1024, 4096] -> [st 8, p 128, a 8, s 512] contiguous
        xt = x[b].T.reshape(8, 128, 8, 512).transpose(2, 1, 0, 3)
        # wv cols / wo rows permuted to the device slot order
        wv = Wv[sl].T[:, _SLOT_PERM]
        wo = Wo[:, sl].T[_SLOT_PERM, :]
        in_maps.append({
            "xt": np.ascontiguousarray(xt).astype(np.float16),
            "wq": _tile_w(Wq[sl].T * SCALE),
            "wk": _tile_w(Wk[sl].T),
            "wv": _tile_w(wv),
            "wo": np.ascontiguousarray(
                wo.reshape(2, 128, 1024).transpose(1, 0, 2)
            ).astype(np.float16),
            "maskin": MI,
        })
    return in_maps


def combine(results, inputs):
    """Sum per-core partials and add host-side corrections."""
    x = np.asarray(inputs["x"], np.float32)
    Wv = np.asarray(inputs["Wv"], np.float32)
    Wo = np.asarray(inputs["Wo"], np.float32)
    bv = np.asarray(inputs["bv"], np.float32)
    bo = np.asarray(inputs["bo"], np.float32)
    out = np.zeros((BATCH, SEQ, D_MODEL), np.float32)
    for core in range(N_CORES):
        # device layout [ib, mh, 128, 512] -> [4096, 1024]
        o = results[core]["out"].astype(np.float32)
        out[core // 4] += o.transpose(0, 2, 1, 3).reshape(SEQ, D_MODEL)
    # reference adds 1e-9 to every attn prob (including masked ones):
    # ctx += 1e-9 * sum_j v[j]  ->  out += 1e-9 * (sum_j v[j]) @ Wo^T
    for b in range(BATCH):
        vs = x[b].sum(axis=0) @ Wv.T + SEQ * bv
        out[b] += (1e-9 * (vs @ Wo.T) + bo)[None, :]
    return out


def run_cores(in_maps, trace=False, **kw):
    from concourse.bass_utils import run_bass_kernel_spmd
    nc = get_program()
    return run_bass_kernel_spmd(nc, in_maps, core_ids=list(range(N_CORES)),
                                trace=trace, **kw)


def kernel(**inputs):
    in_maps = make_in_maps(inputs)
    res = run_cores(in_maps)
    return combine(res.results, inputs)
